# revision 9
# baseline (speedup 1.0000x reference)
"""Inverse DTCWT (biort bandpass) level-1 reconstruction as a Bass/Tile kernel.

Math: the reference is
    y = (A0 @ Yl + A1 @ lh) @ A0^T + (A0 @ hl) @ A1^T + (A2 @ hh) @ A2^T
where A* are 256x256 banded matrices (1D taps + symmetric padding folded in)
and lh/hl/hh are the c2q quad-interleaves of subband pairs (0,5)/(2,3)/(1,4).

Row r of a c2q image comes from `top` (r even) or `bot` (r odd), each a
128x256 column-interleaved image built with 3 DVE tensor-tensor ops per pair:
    top[:, 0::2] = w1r + w2r ; top[:, 1::2] = w1i + w2i
    bot[:, 0::2] = w1i - w2i ; bot[:, 1::2] = w2r - w1r
The row interleave never materializes: contraction over rows splits into
even/odd with host-precomputed matrices Re = A^T[0::2]/sqrt2, Ro = A^T[1::2]/sqrt2.

Stage A (col filters) runs with the *image tiles stationary* producing
transposed intermediates Z[c, h] in PSUM; stage B (row filters) consumes Z
slices as stationary against A^T and accumulates all three paths into one
PSUM bank in natural orientation. No transposes anywhere.

Sharding: pure data parallel, batch dim (8) across 8 cores.
"""
import sys

if "/opt/trn_rl_repo" not in sys.path:
    sys.path.insert(0, "/opt/trn_rl_repo")

import numpy as np

_B, _C, _H, _W = 8, 64, 256, 256
_NCORES = 8
_G = 4  # images (channels) per group
# trim half-widths of the banded matrices (taps//2)
_M0, _M1, _M2 = 6, 9, 6


def _band_matrix(h, N):
    """A @ x == colfilter(x, h) with symmetric padding, in float64."""
    h = np.asarray(h, dtype=np.float64)
    L = h.shape[0]
    m = L // 2
    A = np.zeros((N, N), dtype=np.float64)
    for i in range(N):
        for k in range(L):
            s = i + k - m
            if s < 0:
                s = -1 - s
            elif s >= N:
                s = 2 * N - 1 - s
            A[i, s] += h[L - 1 - k]
    return A


def build_consts(g0o, g1o, g2o):
    """Host-side constant tensors handed to every core."""
    A0 = _band_matrix(g0o, _H).T  # stored transposed: [r, h]
    A1 = _band_matrix(g1o, _H).T
    A2 = _band_matrix(g2o, _H).T
    s2 = np.sqrt(2.0)

    def tile2(AT):  # [256, 256] -> [128, 2, 256] with [p, kr, h] = AT[128*kr+p, h]
        return np.ascontiguousarray(
            AT.reshape(2, 128, 256).transpose(1, 0, 2)
        ).astype(np.float32)

    a0t, a1t, a2t = tile2(A0), tile2(A1), tile2(A2)
    # rmats[q, e/o]: per-pair col-filter matrices; pair q uses bands (q, 5-q):
    #   q=0 (lh)   -> col filter A1 ; q=1 (hh) -> A2 ; q=2 (hl) -> A0
    rmats = np.stack(
        [
            np.stack([A1[0::2] / s2, A1[1::2] / s2]),
            np.stack([A2[0::2] / s2, A2[1::2] / s2]),
            np.stack([A0[0::2] / s2, A0[1::2] / s2]),
        ]
    ).astype(np.float32)  # [3, 2, 128, 256]
    return {"a0t": a0t, "a1t": a1t, "a2t": a2t, "rmats": rmats}


def build_nc(n_images):
    import concourse.bacc as bacc
    import concourse.mybir as mybir
    from concourse.tile import TileContext

    f32 = mybir.dt.float32
    f32r = mybir.dt.float32r
    mm = lambda ap: ap  # tiles are declared float32r already
    nc = bacc.Bacc(None, target_bir_lowering=False, debug=False)

    yl_d = nc.declare_dram_parameter("yl", [n_images, 256, 256], f32r, isOutput=False)
    yhr_d = nc.declare_dram_parameter("yhr", [n_images, 6, 128, 128], f32, isOutput=False)
    yhi_d = nc.declare_dram_parameter("yhi", [n_images, 6, 128, 128], f32, isOutput=False)
    a0t_d = nc.declare_dram_parameter("a0t", [128, 2, 256], f32r, isOutput=False)
    a1t_d = nc.declare_dram_parameter("a1t", [128, 2, 256], f32r, isOutput=False)
    a2t_d = nc.declare_dram_parameter("a2t", [128, 2, 256], f32r, isOutput=False)
    rm_d = nc.declare_dram_parameter("rmats", [3, 2, 128, 256], f32r, isOutput=False)
    out_d = nc.declare_dram_parameter("out", [n_images, 256, 256], f32, isOutput=True)

    n_groups = n_images // _G
    assert n_groups * _G == n_images

    with TileContext(nc) as tc:
        with (
            tc.tile_pool(name="consts", bufs=1) as cpool,
            tc.tile_pool(name="io", bufs=2) as io_pool,
            tc.tile_pool(name="tb", bufs=2) as tb_pool,
            tc.tile_pool(name="zsb", bufs=2) as z_pool,
            tc.tile_pool(name="ps", bufs=2, space="PSUM") as ps_pool,
        ):
            a0t = cpool.tile([128, 2, 256], f32r)
            a1t = cpool.tile([128, 2, 256], f32r)
            a2t = cpool.tile([128, 2, 256], f32r)
            rm = cpool.tile([128, 3, 2, 256], f32r)
            nc.sync.dma_start(a0t[:], a0t_d[:])
            nc.sync.dma_start(a1t[:], a1t_d[:])
            nc.sync.dma_start(a2t[:], a2t_d[:])
            nc.sync.dma_start(rm[:], rm_d[:].rearrange("q e t h -> t q e h"))

            # stage-B row-filter matrix + band half-width per Z path
            stage_b = [(a0t, _M0), (a1t, _M1), (a2t, _M2)]

            for g in range(n_groups):
                c0 = g * _G
                yh = io_pool.tile([128, _G, 6, 2, 128], f32, tag="yh", bufs=3)
                yl = io_pool.tile([128, _G, 2, 256], f32r, tag="yl")
                for q in range(3):
                    for s in (q, 5 - q):  # pair-q bands land together
                        nc.sync.dma_start(
                            yh[:, :, s, 0, :],
                            yhr_d[c0 : c0 + _G, s].rearrange("i h w -> h i w"),
                        )
                        nc.sync.dma_start(
                            yh[:, :, s, 1, :],
                            yhi_d[c0 : c0 + _G, s].rearrange("i h w -> h i w"),
                        )
                nc.sync.dma_start(
                    yl[:],
                    yl_d[c0 : c0 + _G].rearrange("i (k p) w -> p i k w", p=128),
                )

                top = tb_pool.tile([128, _G, 3, 128, 2], f32r, tag="top")
                bot = tb_pool.tile([128, _G, 3, 128, 2], f32r, tag="bot")
                for q in range(3):
                    # all images at once; w1 = band q, w2 = band 5-q
                    w1 = yh[:, :, q, :, :].transpose([0, 1, 3, 2])  # (p, i, w, ri)
                    w2 = yh[:, :, 5 - q, :, :].transpose([0, 1, 3, 2])
                    nc.vector.tensor_add(top[:, :, q, :, :], w1, w2)
                    nc.vector.tensor_sub(
                        bot[:, :, q, :, 0], yh[:, :, q, 1, :], yh[:, :, 5 - q, 1, :]
                    )
                    nc.vector.tensor_sub(
                        bot[:, :, q, :, 1], yh[:, :, 5 - q, 0, :], yh[:, :, q, 0, :]
                    )

                out_sb = io_pool.tile([128, _G, 2, 256], f32, tag="out_sb")
                for i in range(_G):
                    # ---- stage A: Z[c, h] = col-filtered, transposed ----
                    z1 = ps_pool.tile([128, 2, 256], f32, tag="z1")
                    z2 = ps_pool.tile([128, 2, 256], f32, tag="z2")
                    z3 = ps_pool.tile([128, 2, 256], f32, tag="z3")
                    for cc in range(2):
                        js = slice(64 * cc, 64 * cc + 64)
                        ws = slice(128 * cc, 128 * cc + 128)
                        # z1: lh path (pair q=0, col A1) + Yl path (col A0)
                        nc.tensor.matmul(
                            z1[:, cc, :], mm(top[:, i, 0, js, :]), mm(rm[:, 0, 0, :]),
                            start=True, stop=False,
                        )
                        nc.tensor.matmul(
                            z1[:, cc, :], mm(bot[:, i, 0, js, :]), mm(rm[:, 0, 1, :]),
                            start=False, stop=False,
                        )
                        nc.tensor.matmul(
                            z1[:, cc, :], mm(yl[:, i, 0, ws]), mm(a0t[:, 0, :]),
                            start=False, stop=False,
                        )
                        nc.tensor.matmul(
                            z1[:, cc, :], mm(yl[:, i, 1, ws]), mm(a0t[:, 1, :]),
                            start=False, stop=True,
                        )
                        # z2: hl path (pair q=2, col A0); row filter A1 later
                        nc.tensor.matmul(
                            z2[:, cc, :], mm(top[:, i, 2, js, :]), mm(rm[:, 2, 0, :]),
                            start=True, stop=False,
                        )
                        nc.tensor.matmul(
                            z2[:, cc, :], mm(bot[:, i, 2, js, :]), mm(rm[:, 2, 1, :]),
                            start=False, stop=True,
                        )
                        # z3: hh path (pair q=1, col A2); row filter A2 later
                        nc.tensor.matmul(
                            z3[:, cc, :], mm(top[:, i, 1, js, :]), mm(rm[:, 1, 0, :]),
                            start=True, stop=False,
                        )
                        nc.tensor.matmul(
                            z3[:, cc, :], mm(bot[:, i, 1, js, :]), mm(rm[:, 1, 1, :]),
                            start=False, stop=True,
                        )
                    z1s = z_pool.tile([128, 2, 256], f32r, tag="z1s")
                    z2s = z_pool.tile([128, 2, 256], f32r, tag="z2s")
                    z3s = z_pool.tile([128, 2, 256], f32r, tag="z3s")
                    nc.scalar.copy(z1s[:], z1[:])
                    nc.scalar.copy(z2s[:], z2[:])
                    nc.scalar.copy(z3s[:], z3[:])

                    # ---- stage B: y[r, cout] = sum_paths Z^T @ A^T ----
                    yp = ps_pool.tile([128, 2, 256], f32, tag="yp")
                    for r in range(2):
                        rs = slice(128 * r, 128 * r + 128)
                        first = True
                        for zs, (amat, m) in zip((z1s, z2s, z3s), stage_b):
                            nc.tensor.matmul(
                                yp[:, r, :], mm(zs[:, 0, rs]), mm(amat[:, 0, :]),
                                start=first, stop=False,
                            )
                            nc.tensor.matmul(
                                yp[:, r, :], mm(zs[:, 1, rs]), mm(amat[:, 1, :]),
                                start=False, stop=(zs is z3s),
                            )
                            first = False
                    nc.scalar.copy(out_sb[:, i, :, :], yp[:])

                nc.scalar.dma_start(
                    out_d[c0 : c0 + _G].rearrange("i (k p) w -> p i k w", p=128),
                    out_sb[:],
                )
    nc.compile()
    return nc


_NC_CACHE = {}


def _get_nc(n_images):
    if n_images not in _NC_CACHE:
        _NC_CACHE[n_images] = build_nc(n_images)
    return _NC_CACHE[n_images]


def kernel(Yl, Yhr, Yhi, g0o, g1o, g2o):
    from concourse.bass_utils import run_bass_kernel_spmd

    Yl = np.ascontiguousarray(np.asarray(Yl, dtype=np.float32))
    Yhr = np.ascontiguousarray(np.asarray(Yhr, dtype=np.float32))
    Yhi = np.ascontiguousarray(np.asarray(Yhi, dtype=np.float32))
    consts = build_consts(np.asarray(g0o), np.asarray(g1o), np.asarray(g2o))

    nc = _get_nc(_C)
    in_maps = []
    for k in range(_NCORES):
        in_maps.append(
            {
                "yl": Yl[k],
                "yhr": Yhr[k],
                "yhi": Yhi[k],
                **consts,
            }
        )
    res = run_bass_kernel_spmd(nc, in_maps, list(range(_NCORES)))
    out = np.stack([res.results[k]["out"] for k in range(_NCORES)])
    return out.astype(np.float32)


# revision 12
# speedup vs baseline: 1.1038x; 1.1038x over previous
"""Inverse DTCWT (biort bandpass) level-1 reconstruction as a Bass/Tile kernel.

Math: the reference is
    y = (A0 @ Yl + A1 @ lh) @ A0^T + (A0 @ hl) @ A1^T + (A2 @ hh) @ A2^T
where A* are 256x256 banded matrices (1D taps + symmetric padding folded in)
and lh/hl/hh are the c2q quad-interleaves of subband pairs (0,5)/(2,3)/(1,4).

Row r of a c2q image comes from `top` (r even) or `bot` (r odd), each a
128x256 column-interleaved image built with 3 DVE tensor-tensor ops per pair:
    top[:, 0::2] = w1r + w2r ; top[:, 1::2] = w1i + w2i
    bot[:, 0::2] = w1i - w2i ; bot[:, 1::2] = w2r - w1r
The row interleave never materializes: contraction over rows splits into
even/odd with host-precomputed matrices Re = A^T[0::2]/sqrt2, Ro = A^T[1::2]/sqrt2.

Stage A (col filters) runs with the *image tiles stationary* producing
transposed intermediates Z[c, h] in PSUM; stage B (row filters) consumes Z
slices as stationary against A^T and accumulates all three paths into one
PSUM bank in natural orientation. No transposes anywhere.

Sharding: pure data parallel, batch dim (8) across 8 cores.
"""
import sys

if "/opt/trn_rl_repo" not in sys.path:
    sys.path.insert(0, "/opt/trn_rl_repo")

import numpy as np

_B, _C, _H, _W = 8, 64, 256, 256
_NCORES = 8
_G = 4  # images (channels) per group
# trim half-widths of the banded matrices (taps//2)
_M0, _M1, _M2 = 6, 9, 6


def _band_matrix(h, N):
    """A @ x == colfilter(x, h) with symmetric padding, in float64."""
    h = np.asarray(h, dtype=np.float64)
    L = h.shape[0]
    m = L // 2
    A = np.zeros((N, N), dtype=np.float64)
    for i in range(N):
        for k in range(L):
            s = i + k - m
            if s < 0:
                s = -1 - s
            elif s >= N:
                s = 2 * N - 1 - s
            A[i, s] += h[L - 1 - k]
    return A


def build_consts(g0o, g1o, g2o):
    """Host-side constant tensors handed to every core."""
    A0 = _band_matrix(g0o, _H).T  # stored transposed: [r, h]
    A1 = _band_matrix(g1o, _H).T
    A2 = _band_matrix(g2o, _H).T
    s2 = np.sqrt(2.0)

    def tile2(AT):  # [256, 256] -> [128, 2, 256] with [p, kr, h] = AT[128*kr+p, h]
        return np.ascontiguousarray(
            AT.reshape(2, 128, 256).transpose(1, 0, 2)
        ).astype(np.float32)

    a0t, a1t, a2t = tile2(A0), tile2(A1), tile2(A2)
    # rmats[q, e/o]: per-pair col-filter matrices; pair q uses bands (q, 5-q):
    #   q=0 (lh)   -> col filter A1 ; q=1 (hh) -> A2 ; q=2 (hl) -> A0
    rmats = np.stack(
        [
            np.stack([A1[0::2] / s2, A1[1::2] / s2]),
            np.stack([A2[0::2] / s2, A2[1::2] / s2]),
            np.stack([A0[0::2] / s2, A0[1::2] / s2]),
        ]
    ).astype(np.float32)  # [3, 2, 128, 256]
    return {"a0t": a0t, "a1t": a1t, "a2t": a2t, "rmats": rmats}


def build_nc(n_images):
    import concourse.bacc as bacc
    import concourse.mybir as mybir
    from concourse.tile import TileContext

    f32 = mybir.dt.float32
    f32r = mybir.dt.float32r
    mm = lambda ap: ap  # tiles are declared float32r already
    nc = bacc.Bacc(None, target_bir_lowering=False, debug=False)

    yl_d = nc.declare_dram_parameter("yl", [n_images, 256, 256], f32r, isOutput=False)
    yhr_d = nc.declare_dram_parameter("yhr", [n_images, 6, 128, 128], f32, isOutput=False)
    yhi_d = nc.declare_dram_parameter("yhi", [n_images, 6, 128, 128], f32, isOutput=False)
    a0t_d = nc.declare_dram_parameter("a0t", [128, 2, 256], f32r, isOutput=False)
    a1t_d = nc.declare_dram_parameter("a1t", [128, 2, 256], f32r, isOutput=False)
    a2t_d = nc.declare_dram_parameter("a2t", [128, 2, 256], f32r, isOutput=False)
    rm_d = nc.declare_dram_parameter("rmats", [3, 2, 128, 256], f32r, isOutput=False)
    out_d = nc.declare_dram_parameter("out", [n_images, 256, 256], f32, isOutput=True)

    n_groups = n_images // _G
    assert n_groups * _G == n_images

    with TileContext(nc) as tc:
        with (
            tc.tile_pool(name="consts", bufs=1) as cpool,
            tc.tile_pool(name="io", bufs=2) as io_pool,
            tc.tile_pool(name="tb", bufs=2) as tb_pool,
            tc.tile_pool(name="zsb", bufs=2) as z_pool,
            tc.tile_pool(name="ps", bufs=2, space="PSUM") as ps_pool,
        ):
            a0t = cpool.tile([128, 2, 256], f32r)
            a1t = cpool.tile([128, 2, 256], f32r)
            a2t = cpool.tile([128, 2, 256], f32r)
            rm = cpool.tile([128, 3, 2, 256], f32r)
            nc.sync.dma_start(a0t[:], a0t_d[:])
            nc.sync.dma_start(a1t[:], a1t_d[:])
            nc.sync.dma_start(a2t[:], a2t_d[:])
            nc.sync.dma_start(rm[:], rm_d[:].rearrange("q e t h -> t q e h"))

            # stage-B row-filter matrix + band half-width per Z path
            stage_b = [(a0t, _M0), (a1t, _M1), (a2t, _M2)]

            for g in range(n_groups):
                c0 = g * _G
                yh = io_pool.tile([128, _G, 6, 2, 128], f32, tag="yh")
                yl = io_pool.tile([128, _G, 2, 256], f32r, tag="yl")
                nc.sync.dma_start(
                    yh[:, :, :, 0, :],
                    yhr_d[c0 : c0 + _G].rearrange("i s h w -> h i s w"),
                )
                nc.sync.dma_start(
                    yh[:, :, :, 1, :],
                    yhi_d[c0 : c0 + _G].rearrange("i s h w -> h i s w"),
                )
                nc.sync.dma_start(
                    yl[:],
                    yl_d[c0 : c0 + _G].rearrange("i (k p) w -> p i k w", p=128),
                )

                top = tb_pool.tile([128, _G, 3, 128, 2], f32r, tag="top", bufs=3)
                bot = tb_pool.tile([128, _G, 3, 128, 2], f32r, tag="bot", bufs=3)
                for q in range(3):
                    # all images at once; w1 = band q, w2 = band 5-q
                    w1 = yh[:, :, q, :, :].transpose([0, 1, 3, 2])  # (p, i, w, ri)
                    w2 = yh[:, :, 5 - q, :, :].transpose([0, 1, 3, 2])
                    nc.vector.tensor_add(top[:, :, q, :, :], w1, w2)
                    nc.gpsimd.tensor_sub(
                        bot[:, :, q, :, 0], yh[:, :, q, 1, :], yh[:, :, 5 - q, 1, :]
                    )
                    nc.gpsimd.tensor_sub(
                        bot[:, :, q, :, 1], yh[:, :, 5 - q, 0, :], yh[:, :, q, 0, :]
                    )

                out_sb = io_pool.tile([128, _G, 2, 256], f32, tag="out_sb")
                for i in range(_G):
                    # ---- stage A: Z[c, h] = col-filtered, transposed ----
                    z1 = ps_pool.tile([128, 2, 256], f32, tag="z1")
                    z2 = ps_pool.tile([128, 2, 256], f32, tag="z2")
                    z3 = ps_pool.tile([128, 2, 256], f32, tag="z3")
                    for cc in range(2):
                        js = slice(64 * cc, 64 * cc + 64)
                        ws = slice(128 * cc, 128 * cc + 128)
                        # z1: lh path (pair q=0, col A1) + Yl path (col A0)
                        nc.tensor.matmul(
                            z1[:, cc, :], mm(top[:, i, 0, js, :]), mm(rm[:, 0, 0, :]),
                            start=True, stop=False,
                        )
                        nc.tensor.matmul(
                            z1[:, cc, :], mm(bot[:, i, 0, js, :]), mm(rm[:, 0, 1, :]),
                            start=False, stop=False,
                        )
                        nc.tensor.matmul(
                            z1[:, cc, :], mm(yl[:, i, 0, ws]), mm(a0t[:, 0, :]),
                            start=False, stop=False,
                        )
                        nc.tensor.matmul(
                            z1[:, cc, :], mm(yl[:, i, 1, ws]), mm(a0t[:, 1, :]),
                            start=False, stop=True,
                        )
                        # z2: hl path (pair q=2, col A0); row filter A1 later
                        nc.tensor.matmul(
                            z2[:, cc, :], mm(top[:, i, 2, js, :]), mm(rm[:, 2, 0, :]),
                            start=True, stop=False,
                        )
                        nc.tensor.matmul(
                            z2[:, cc, :], mm(bot[:, i, 2, js, :]), mm(rm[:, 2, 1, :]),
                            start=False, stop=True,
                        )
                        # z3: hh path (pair q=1, col A2); row filter A2 later
                        nc.tensor.matmul(
                            z3[:, cc, :], mm(top[:, i, 1, js, :]), mm(rm[:, 1, 0, :]),
                            start=True, stop=False,
                        )
                        nc.tensor.matmul(
                            z3[:, cc, :], mm(bot[:, i, 1, js, :]), mm(rm[:, 1, 1, :]),
                            start=False, stop=True,
                        )
                    z1s = z_pool.tile([128, 2, 256], f32r, tag="z1s")
                    z2s = z_pool.tile([128, 2, 256], f32r, tag="z2s")
                    z3s = z_pool.tile([128, 2, 256], f32r, tag="z3s")
                    nc.scalar.copy(z1s[:], z1[:])
                    nc.scalar.copy(z2s[:], z2[:])
                    nc.scalar.copy(z3s[:], z3[:])

                    # ---- stage B: y[r, cout] = sum_paths Z^T @ A^T ----
                    yp = ps_pool.tile([128, 2, 256], f32, tag="yp")
                    for r in range(2):
                        rs = slice(128 * r, 128 * r + 128)
                        first = True
                        for zs, (amat, m) in zip((z1s, z2s, z3s), stage_b):
                            nc.tensor.matmul(
                                yp[:, r, :], mm(zs[:, 0, rs]), mm(amat[:, 0, :]),
                                start=first, stop=False,
                            )
                            nc.tensor.matmul(
                                yp[:, r, :], mm(zs[:, 1, rs]), mm(amat[:, 1, :]),
                                start=False, stop=(zs is z3s),
                            )
                            first = False
                    nc.scalar.copy(out_sb[:, i, :, :], yp[:])

                nc.scalar.dma_start(
                    out_d[c0 : c0 + _G].rearrange("i (k p) w -> p i k w", p=128),
                    out_sb[:],
                )
    nc.compile()
    return nc


_NC_CACHE = {}


def _get_nc(n_images):
    if n_images not in _NC_CACHE:
        _NC_CACHE[n_images] = build_nc(n_images)
    return _NC_CACHE[n_images]


def kernel(Yl, Yhr, Yhi, g0o, g1o, g2o):
    from concourse.bass_utils import run_bass_kernel_spmd

    Yl = np.ascontiguousarray(np.asarray(Yl, dtype=np.float32))
    Yhr = np.ascontiguousarray(np.asarray(Yhr, dtype=np.float32))
    Yhi = np.ascontiguousarray(np.asarray(Yhi, dtype=np.float32))
    consts = build_consts(np.asarray(g0o), np.asarray(g1o), np.asarray(g2o))

    nc = _get_nc(_C)
    in_maps = []
    for k in range(_NCORES):
        in_maps.append(
            {
                "yl": Yl[k],
                "yhr": Yhr[k],
                "yhi": Yhi[k],
                **consts,
            }
        )
    res = run_bass_kernel_spmd(nc, in_maps, list(range(_NCORES)))
    out = np.stack([res.results[k]["out"] for k in range(_NCORES)])
    return out.astype(np.float32)


# revision 14
# speedup vs baseline: 1.1160x; 1.0111x over previous
"""Inverse DTCWT (biort bandpass) level-1 reconstruction as a Bass/Tile kernel.

Math: the reference is
    y = (A0 @ Yl + A1 @ lh) @ A0^T + (A0 @ hl) @ A1^T + (A2 @ hh) @ A2^T
where A* are 256x256 banded matrices (1D taps + symmetric padding folded in)
and lh/hl/hh are the c2q quad-interleaves of subband pairs (0,5)/(2,3)/(1,4).

Row r of a c2q image comes from `top` (r even) or `bot` (r odd), each a
128x256 column-interleaved image built with 3 DVE tensor-tensor ops per pair:
    top[:, 0::2] = w1r + w2r ; top[:, 1::2] = w1i + w2i
    bot[:, 0::2] = w1i - w2i ; bot[:, 1::2] = w2r - w1r
The row interleave never materializes: contraction over rows splits into
even/odd with host-precomputed matrices Re = A^T[0::2]/sqrt2, Ro = A^T[1::2]/sqrt2.

Stage A (col filters) runs with the *image tiles stationary* producing
transposed intermediates Z[c, h] in PSUM; stage B (row filters) consumes Z
slices as stationary against A^T and accumulates all three paths into one
PSUM bank in natural orientation. No transposes anywhere.

Sharding: pure data parallel, batch dim (8) across 8 cores.
"""
import sys

if "/opt/trn_rl_repo" not in sys.path:
    sys.path.insert(0, "/opt/trn_rl_repo")

import numpy as np

_B, _C, _H, _W = 8, 64, 256, 256
_NCORES = 8
_G = 4  # images (channels) per group
# trim half-widths of the banded matrices (taps//2)
_M0, _M1, _M2 = 6, 9, 6


def _band_matrix(h, N):
    """A @ x == colfilter(x, h) with symmetric padding, in float64."""
    h = np.asarray(h, dtype=np.float64)
    L = h.shape[0]
    m = L // 2
    A = np.zeros((N, N), dtype=np.float64)
    for i in range(N):
        for k in range(L):
            s = i + k - m
            if s < 0:
                s = -1 - s
            elif s >= N:
                s = 2 * N - 1 - s
            A[i, s] += h[L - 1 - k]
    return A


def build_consts(g0o, g1o, g2o):
    """Host-side constant tensors handed to every core."""
    A0 = _band_matrix(g0o, _H).T  # stored transposed: [r, h]
    A1 = _band_matrix(g1o, _H).T
    A2 = _band_matrix(g2o, _H).T
    s2 = np.sqrt(2.0)

    def tile2(AT):  # [256, 256] -> [128, 2, 256] with [p, kr, h] = AT[128*kr+p, h]
        return np.ascontiguousarray(
            AT.reshape(2, 128, 256).transpose(1, 0, 2)
        ).astype(np.float32)

    a0t, a1t, a2t = tile2(A0), tile2(A1), tile2(A2)
    # rmats[q, e/o]: per-pair col-filter matrices; pair q uses bands (q, 5-q):
    #   q=0 (lh)   -> col filter A1 ; q=1 (hh) -> A2 ; q=2 (hl) -> A0
    rmats = np.stack(
        [
            np.stack([A1[0::2] / s2, A1[1::2] / s2]),
            np.stack([A2[0::2] / s2, A2[1::2] / s2]),
            np.stack([A0[0::2] / s2, A0[1::2] / s2]),
        ]
    ).astype(np.float32)  # [3, 2, 128, 256]
    return {"a0t": a0t, "a1t": a1t, "a2t": a2t, "rmats": rmats}


def build_nc(n_images):
    import concourse.bacc as bacc
    import concourse.mybir as mybir
    from concourse.tile import TileContext

    f32 = mybir.dt.float32
    f32r = mybir.dt.float32r
    mm = lambda ap: ap  # tiles are declared float32r already
    nc = bacc.Bacc(None, target_bir_lowering=False, debug=False)

    yl_d = nc.declare_dram_parameter("yl", [n_images, 256, 256], f32r, isOutput=False)
    yhr_d = nc.declare_dram_parameter("yhr", [n_images, 6, 128, 128], f32, isOutput=False)
    yhi_d = nc.declare_dram_parameter("yhi", [n_images, 6, 128, 128], f32, isOutput=False)
    a0t_d = nc.declare_dram_parameter("a0t", [128, 2, 256], f32r, isOutput=False)
    a1t_d = nc.declare_dram_parameter("a1t", [128, 2, 256], f32r, isOutput=False)
    a2t_d = nc.declare_dram_parameter("a2t", [128, 2, 256], f32r, isOutput=False)
    rm_d = nc.declare_dram_parameter("rmats", [3, 2, 128, 256], f32r, isOutput=False)
    out_d = nc.declare_dram_parameter("out", [n_images, 256, 256], f32, isOutput=True)

    n_groups = n_images // _G
    assert n_groups * _G == n_images

    with TileContext(nc) as tc:
        with (
            tc.tile_pool(name="consts", bufs=1) as cpool,
            tc.tile_pool(name="io", bufs=2) as io_pool,
            tc.tile_pool(name="tb", bufs=2) as tb_pool,
            tc.tile_pool(name="zsb", bufs=2) as z_pool,
            tc.tile_pool(name="ps", bufs=2, space="PSUM") as ps_pool,
        ):
            a0t = cpool.tile([128, 2, 256], f32r)
            a1t = cpool.tile([128, 2, 256], f32r)
            a2t = cpool.tile([128, 2, 256], f32r)
            rm = cpool.tile([128, 3, 2, 256], f32r)
            nc.sync.dma_start(a0t[:], a0t_d[:])
            nc.sync.dma_start(a1t[:], a1t_d[:])
            nc.sync.dma_start(a2t[:], a2t_d[:])
            nc.sync.dma_start(rm[:], rm_d[:].rearrange("q e t h -> t q e h"))

            # stage-B row-filter matrix + band half-width per Z path
            stage_b = [(a0t, _M0), (a1t, _M1), (a2t, _M2)]

            for g in range(n_groups):
                c0 = g * _G
                yh = io_pool.tile([128, _G, 6, 2, 128], f32, tag="yh", bufs=3)
                yl = io_pool.tile([128, _G, 2, 256], f32r, tag="yl")
                nc.sync.dma_start(
                    yh[:, :, :, 0, :],
                    yhr_d[c0 : c0 + _G].rearrange("i s h w -> h i s w"),
                )
                nc.sync.dma_start(
                    yh[:, :, :, 1, :],
                    yhi_d[c0 : c0 + _G].rearrange("i s h w -> h i s w"),
                )
                nc.sync.dma_start(
                    yl[:],
                    yl_d[c0 : c0 + _G].rearrange("i (k p) w -> p i k w", p=128),
                )

                top = tb_pool.tile([128, _G, 3, 128, 2], f32r, tag="top", bufs=3)
                bot = tb_pool.tile([128, _G, 3, 128, 2], f32r, tag="bot", bufs=3)
                for q in range(3):
                    # all images at once; w1 = band q, w2 = band 5-q
                    w1 = yh[:, :, q, :, :].transpose([0, 1, 3, 2])  # (p, i, w, ri)
                    w2 = yh[:, :, 5 - q, :, :].transpose([0, 1, 3, 2])
                    nc.vector.tensor_add(top[:, :, q, :, :], w1, w2)
                    nc.vector.tensor_sub(
                        bot[:, :, q, :, 0], yh[:, :, q, 1, :], yh[:, :, 5 - q, 1, :]
                    )
                    nc.vector.tensor_sub(
                        bot[:, :, q, :, 1], yh[:, :, 5 - q, 0, :], yh[:, :, q, 0, :]
                    )

                out_sb = io_pool.tile([128, _G, 2, 256], f32, tag="out_sb")
                for i in range(_G):
                    # ---- stage A: Z[c, h] = col-filtered, transposed ----
                    z1 = ps_pool.tile([128, 2, 256], f32, tag="z1")
                    z2 = ps_pool.tile([128, 2, 256], f32, tag="z2")
                    z3 = ps_pool.tile([128, 2, 256], f32, tag="z3")
                    for cc in range(2):
                        js = slice(64 * cc, 64 * cc + 64)
                        ws = slice(128 * cc, 128 * cc + 128)
                        # z1: lh path (pair q=0, col A1) + Yl path (col A0)
                        nc.tensor.matmul(
                            z1[:, cc, :], mm(top[:, i, 0, js, :]), mm(rm[:, 0, 0, :]),
                            start=True, stop=False,
                        )
                        nc.tensor.matmul(
                            z1[:, cc, :], mm(bot[:, i, 0, js, :]), mm(rm[:, 0, 1, :]),
                            start=False, stop=False,
                        )
                        nc.tensor.matmul(
                            z1[:, cc, :], mm(yl[:, i, 0, ws]), mm(a0t[:, 0, :]),
                            start=False, stop=False,
                        )
                        nc.tensor.matmul(
                            z1[:, cc, :], mm(yl[:, i, 1, ws]), mm(a0t[:, 1, :]),
                            start=False, stop=True,
                        )
                        # z2: hl path (pair q=2, col A0); row filter A1 later
                        nc.tensor.matmul(
                            z2[:, cc, :], mm(top[:, i, 2, js, :]), mm(rm[:, 2, 0, :]),
                            start=True, stop=False,
                        )
                        nc.tensor.matmul(
                            z2[:, cc, :], mm(bot[:, i, 2, js, :]), mm(rm[:, 2, 1, :]),
                            start=False, stop=True,
                        )
                        # z3: hh path (pair q=1, col A2); row filter A2 later
                        nc.tensor.matmul(
                            z3[:, cc, :], mm(top[:, i, 1, js, :]), mm(rm[:, 1, 0, :]),
                            start=True, stop=False,
                        )
                        nc.tensor.matmul(
                            z3[:, cc, :], mm(bot[:, i, 1, js, :]), mm(rm[:, 1, 1, :]),
                            start=False, stop=True,
                        )
                    z1s = z_pool.tile([128, 2, 256], f32r, tag="z1s")
                    z2s = z_pool.tile([128, 2, 256], f32r, tag="z2s")
                    z3s = z_pool.tile([128, 2, 256], f32r, tag="z3s")
                    nc.scalar.copy(z1s[:], z1[:])
                    nc.scalar.copy(z2s[:], z2[:])
                    nc.scalar.copy(z3s[:], z3[:])

                    # ---- stage B: y[r, cout] = sum_paths Z^T @ A^T ----
                    yp = ps_pool.tile([128, 2, 256], f32, tag="yp")
                    for r in range(2):
                        rs = slice(128 * r, 128 * r + 128)
                        first = True
                        for zs, (amat, m) in zip((z1s, z2s, z3s), stage_b):
                            nc.tensor.matmul(
                                yp[:, r, :], mm(zs[:, 0, rs]), mm(amat[:, 0, :]),
                                start=first, stop=False,
                            )
                            nc.tensor.matmul(
                                yp[:, r, :], mm(zs[:, 1, rs]), mm(amat[:, 1, :]),
                                start=False, stop=(zs is z3s),
                            )
                            first = False
                    nc.scalar.copy(out_sb[:, i, :, :], yp[:])

                nc.scalar.dma_start(
                    out_d[c0 : c0 + _G].rearrange("i (k p) w -> p i k w", p=128),
                    out_sb[:],
                )
    nc.compile()
    return nc


_NC_CACHE = {}


def _get_nc(n_images):
    if n_images not in _NC_CACHE:
        _NC_CACHE[n_images] = build_nc(n_images)
    return _NC_CACHE[n_images]


def kernel(Yl, Yhr, Yhi, g0o, g1o, g2o):
    from concourse.bass_utils import run_bass_kernel_spmd

    Yl = np.ascontiguousarray(np.asarray(Yl, dtype=np.float32))
    Yhr = np.ascontiguousarray(np.asarray(Yhr, dtype=np.float32))
    Yhi = np.ascontiguousarray(np.asarray(Yhi, dtype=np.float32))
    consts = build_consts(np.asarray(g0o), np.asarray(g1o), np.asarray(g2o))

    nc = _get_nc(_C)
    in_maps = []
    for k in range(_NCORES):
        in_maps.append(
            {
                "yl": Yl[k],
                "yhr": Yhr[k],
                "yhi": Yhi[k],
                **consts,
            }
        )
    res = run_bass_kernel_spmd(nc, in_maps, list(range(_NCORES)))
    out = np.stack([res.results[k]["out"] for k in range(_NCORES)])
    return out.astype(np.float32)
